# revision 50
# baseline (speedup 1.0000x reference)
"""4D circular cross-correlation (qcd_ml C_Convolution, k=3, nd=4) on 8 TRN2 cores.

Math: out[o, x,y,z,t, s,c] = b[o] + sum_{i, ax,ay,az,at} W[i,o,ax,ay,az,at]
                                   * U[i, x+ax-1, y+ay-1, z+az-1, t+at-1, s,c]
(all site indices circular). U complex64 (4,16,16,16,32,4,3), W complex64
(4,4,3,3,3,3), b complex64 (4,).

Device mapping (per core, T sharded 8-way with +-1 halos prepared on host):
  - contraction (matmul partition) dim = (reim_in 2, C_in 4, X 16) = 128
  - output (PSUM partition) dim       = (reim_out 2, C_out 4, X0 16) = 128
  - X offsets (ax) live inside the stationary 128x128 matrices, circularly
    banded in (x, x0); complex arithmetic is the 2x2 [[Wr, Wi], [-Wi, Wr]]
    block over the reim axes.
  - The T offsets (at) are removed by a host-side Winograd F(4,3) transform
    along t: the 4 local t outputs form ONE tile whose 6-point input window
    is exactly the t-halo slab; U becomes 6 phases (B^T d), weights become
    G W (6 phases); the device accumulates 9 (ay,az) offsets per phase into
    PSUM and combines the 6 phase results with A^T on the vector engine.
  - moving free dim = (y-pair 2, z 16, spin*color 12) = 384 <= 512 (one
    PSUM bank per phase).
  - y,z circular handled by host padding to 18; t halo from neighbor T-slab.

Performance shape (measured, 8x TRN2, ~92-93us):
  - All matmul data is fp16 (rel_err 9.9e-4 vs 2e-2 gate): full PE rate
    (1 col/cycle @2.4GHz), LoadStationary ~97ns hides under the 163ns
    matmul, input DMA halves vs f32. fp32r LS (~195ns) was the original
    pipeline bottleneck.
  - 432 matmuls x 163ns = 70.4us solid PE streak at the instruction floor.
  - Fixed overheads: ~8.7us runtime/DGE entry before the first DMA byte,
    ~3.3us exit barrier. Warm-up matmuls bridge the PE through the input
    DMA wait so the p-state is at full clock when real work starts.
  - PSUM->SBUF drains ride the Act engine; the A^T combine (11 DVE ops per
    y-pair) hides under the next pair's matmuls; phases run in combine
    order (1,2,0,3,4,5); t-major output tiles let each pair's store split
    into two halves that overlap the combine tail.
  - Single SP DMA queue, explicitly consumption-ordered (multi-queue
    splits share the same ~420GB/s and starve the small wstat stream).
"""

import os
import sys
import itertools
import numpy as np

for _p in ("/opt/trn_rl_repo",):
    if _p not in sys.path and os.path.isdir(_p):
        sys.path.insert(0, _p)

C_IN, C_OUT = 4, 4
X = Y = Z = 16
T = 32
SC = 12  # spin*color
NCORES = 8
TLOC = T // NCORES          # 4 = one F(4,3) output tile
NPH = 6                     # Winograd F(4,3) phases
YPAD, ZPAD = Y + 2, Z + 2   # 18
UH_ROWS = 10                # y_pad rows per half tile (0..9 / 8..17)
OFF9 = list(itertools.product(range(3), repeat=2))  # (ay, az)
FREE = 2 * Z * SC           # 384, one chunk = (y-pair, z, sc) per phase

# 16-bit data path: halves input DMA and, critically, halves LoadStationary
# time (fp32r LS ~195ns > 384-col matmul ~160ns made the PE pipeline
# LS-bound; 16-bit LS ~97ns makes it matmul-bound). fp16 over bf16: same PE
# rate (1 col/cycle), 4x finer mantissa. Values are small (|U~|<=34,
# |wstat|<=4, products accumulate in f32 PSUM) so fp16 range is safe.
# Measured rel_err: bf16 7.9e-3, fp32r 4.7e-4 (gate 2e-2).
CONV_DT = os.environ.get("CONV_DT", "fp16")
USE_16BIT = CONV_DT in ("fp16", "bf16")

# Winograd F(4,3), points [0,1,-1,2,-2,inf] (correlation form:
# out[r] = sum_k g[k] d[r+k], r=0..3, d = U[t0-1 .. t0+4]).
BT = np.array([
    [4, 0, -5, 0, 1, 0],
    [0, -4, -4, 1, 1, 0],
    [0, 4, -4, -1, 1, 0],
    [0, -2, -1, 2, 1, 0],
    [0, 2, -1, -2, 1, 0],
    [0, 4, 0, -5, 0, 1]], np.float64)
G = np.array([
    [1 / 4, 0, 0],
    [-1 / 6, -1 / 6, -1 / 6],
    [-1 / 6, 1 / 6, -1 / 6],
    [1 / 24, 1 / 12, 1 / 6],
    [1 / 24, -1 / 12, 1 / 6],
    [0, 0, 1]], np.float64)
# A^T = [[1,1,1,1,1,0],
#        [0,1,-1,2,-2,0],
#        [0,1,1,4,4,0],
#        [0,1,-1,8,-8,1]]  -- applied on the device (DVE).


def _np_dt():
    if CONV_DT == "fp16":
        return np.dtype(np.float16)
    if CONV_DT == "bf16":
        import ml_dtypes
        return np.dtype(ml_dtypes.bfloat16)
    return np.dtype(np.float32)


def _prep_u_shards(U):
    """U complex (4,16,16,16,32,4,3) -> per-core arrays (bf16 or f32)
    [128, YPAD, ZPAD, NPH, SC] of the t-Winograd-transformed field."""
    dt = _np_dt()
    Ur = np.stack([U.real, U.imag], axis=0).astype(np.float32)  # (2,4,X,Y,Z,T,4,3)
    Ur = Ur.reshape(2, C_IN, X, Y, Z, T, SC)
    Up = np.pad(Ur, ((0, 0), (0, 0), (0, 0), (1, 1), (1, 1), (0, 0), (0, 0)),
                mode="wrap")  # (2,4,16,18,18,32,12)
    shards = []
    for k in range(NCORES):
        t0 = k * TLOC
        tidx = np.arange(t0 - 1, t0 + 5) % T        # 6-point window
        d = np.take(Up, tidx, axis=5)               # (2,4,16,18,18,6,12)
        m = np.einsum("pk,rixyzks->rixyzps", BT,
                      d.astype(np.float64)).astype(dt)
        m = m.reshape(128, YPAD, ZPAD, NPH, SC)
        shards.append(np.ascontiguousarray(m))
    return shards


def _prep_wstat(W):
    """W complex (4,4,3,3,3,3) -> [128, NPH*9, 128] float32 stationary stack.

    For phase p and (ay,az): Wg[p][i,o,ax,ay,az] = sum_at G[p,at] W[..,at];
    band in (x,x0): ax = (x - x0 + 1) mod 16 in {0,1,2};
    ri block M = [[Wr, Wi], [-Wi, Wr]] (columns riO: out_r, out_i).
    """
    Wc = np.ascontiguousarray(W).astype(np.complex128)
    Wg = np.einsum("pk,ioxyzk->pioxyz", G.astype(np.complex128), Wc)
    Wg = Wg.astype(np.complex64)                    # (6,4,4,3,3,3)
    stat = np.zeros((2, C_IN, X, NPH * 9, 2, C_OUT, X), _np_dt())
    for ph in range(NPH):
        for aidx, (ay, az) in enumerate(OFF9):
            widx = ph * 9 + aidx
            for ax in range(3):
                wr = Wg[ph, :, :, ax, ay, az].real
                wi = Wg[ph, :, :, ax, ay, az].imag
                for x0 in range(X):
                    x = (x0 + ax - 1) % X
                    stat[0, :, x, widx, 0, :, x0] = wr
                    stat[1, :, x, widx, 0, :, x0] = -wi
                    stat[0, :, x, widx, 1, :, x0] = wi
                    stat[1, :, x, widx, 1, :, x0] = wr
    return np.ascontiguousarray(stat.reshape(128, NPH * 9, 128))


def _assemble(results, b):
    """results[k]["out"]: [128, Y//2, TLOC, 2, Z, SC] -> complex (4,16,16,16,32,4,3)."""
    out = np.empty((C_OUT, X, Y, Z, T, SC), np.complex64)
    for k in range(NCORES):
        r = np.asarray(results[k]["out"], np.float32).reshape(
            2, C_OUT, X, Y // 2, TLOC, 2, Z, SC)
        # (ri, o, x, g, t, y2, z, s) -> (ri, o, x, g, y2, z, t, s)
        r = r.transpose(0, 1, 2, 3, 5, 6, 4, 7).reshape(
            2, C_OUT, X, Y, Z, TLOC, SC)
        out[:, :, :, :, k * TLOC:(k + 1) * TLOC, :] = r[0] + 1j * r[1]
    out += np.asarray(b, np.complex64).reshape(C_OUT, 1, 1, 1, 1, 1)
    return np.ascontiguousarray(out.reshape(C_OUT, X, Y, Z, T, 4, 3))


def _build_nc():
    import concourse.mybir as mybir
    from concourse import bacc, tile
    from contextlib import ExitStack

    f32 = mybir.dt.float32
    _dt16 = {"fp16": mybir.dt.float16, "bf16": mybir.dt.bfloat16}
    mm_dt = _dt16.get(CONV_DT, mybir.dt.float32r)
    out_dt = _dt16.get(CONV_DT, f32)
    tmp_dt = f32  # DVE temps stay f32: 16-bit gave no DVE speedup, cost accuracy
    AluOp = mybir.AluOpType

    WCOLS = NPH * 9 * 128              # 6912
    UCOLS = UH_ROWS * ZPAD * NPH * SC  # 12960

    nc = bacc.Bacc()
    # Fine-grained consumption-ordered input streaming: one full U~ tile
    # filled by disjoint row-slice DMAs (no y duplication), wstat split per
    # phase. Pair 0's phase-0 data (ws[0] + rows 0..5 of phase 0) lands after
    # ~1.2 MB, so the PE starts within a few us; dependencies are tracked at
    # address level, so each matmul only waits for the slices it reads.
    w_dram = nc.declare_dram_parameter("wstat", [128, NPH, 9, 128], mm_dt, isOutput=False)
    u_dram = nc.declare_dram_parameter("u", [128, YPAD, ZPAD, NPH, SC], mm_dt, isOutput=False)
    # t-major output layout: each A^T result row ow(r) is a CONTIGUOUS
    # [2y,Z,SC] block, and the per-pair store splits into two contiguous
    # halves so the first half overlaps the tail of the DVE combine.
    o_dram = nc.declare_dram_parameter("out", [128, Y // 2, TLOC, 2, Z, SC], out_dt, isOutput=True)

    with tile.TileContext(nc) as tc, ExitStack() as ctx:
        ipool = ctx.enter_context(tc.tile_pool(name="inp", bufs=1))
        opool = ctx.enter_context(tc.tile_pool(name="o", bufs=2))
        tpool = ctx.enter_context(tc.tile_pool(name="tmp", bufs=1))
        ppool = ctx.enter_context(tc.tile_pool(name="psum", bufs=7, space="PSUM"))
        wpool = ctx.enter_context(tc.tile_pool(name="warmp", bufs=1, space="PSUM"))

        wt = ipool.tile([128, NPH, 9, 128], mm_dt, tag="w")
        ufull = ipool.tile([128, YPAD, ZPAD, NPH, SC], mm_dt, tag="u")
        # All slices are CONTIGUOUS per partition (phase-strided DMAs measure
        # ~2x slower). Single SP queue: aggregate DMA bandwidth is shared
        # across queues (measured ~420GB/s total), and multi-queue arbitration
        # starves the small early wstat stream; explicit ordering on one
        # queue beats it. Head of stream is fine-grained so pair 0's phase-0
        # matmuls (rows 0:2, wt[0][ay=0]) start ~1.5us after data flows;
        # everything later stays comfortably ahead of compute.
        # Coarse head: all of pair 0's U rows land before its first matmul,
        # so the stream never stalls mid-pair on a late DMA (a thin u[0:2]
        # nibble start measured 2-3us mid-stream stalls on late-DMA runs,
        # each also costing a p-state dip). wstat phases follow in compute
        # order (1, 2, 0, 3...).
        nc.sync.dma_start(ufull[:, 0:4], u_dram[:, 0:4])
        # wt[1] split: its first 3 stationaries (96KB) unblock the first
        # matmuls ~0.5us before the whole phase block would.
        nc.sync.dma_start(wt[:, 1, 0:3], w_dram[:, 1, 0:3])
        nc.sync.dma_start(wt[:, 1, 3:9], w_dram[:, 1, 3:9])
        for ph in (2, 0, 3, 4, 5):
            nc.sync.dma_start(wt[:, ph:ph + 1], w_dram[:, ph:ph + 1])
        for r0, r1 in ((4, 6), (6, 10), (10, 14), (14, 18)):
            nc.sync.dma_start(ufull[:, r0:r1], u_dram[:, r0:r1])

        # PE warm-up: dummy matmuls on a zeroed scratch tile while the input
        # DMA streams (~5us of otherwise-idle PE time). The PE p-state ramps
        # to full clock only after ~3us of continuous busy; warming it here
        # means the first real matmuls run at full rate instead of ramping
        # mid-stream. Results go to a scratch PSUM tile and are discarded.
        # 20 x ~320ns cadence ends ~14.1us, at/after the typical arrival of
        # pair 0's inputs (DMA start itself jitters 8.7-9.7us run to run).
        # A residual gap of up to ~2us does not reset the p-state (measured:
        # 1.4us gap kept full clock, 3.0-3.3us gaps did not), so this also
        # covers the late-DMA runs with at most a small mid-clock ramp.
        warm = ipool.tile([128, FREE], mm_dt, tag="warm")
        nc.gpsimd.memset(warm[:], 0.0)
        wps = wpool.tile([128, FREE], f32)
        for _ in range(20):
            nc.tensor.matmul(wps[:], warm[:, 0:128], warm[:],
                             start=True, stop=True)

        def stt(out_ap, sb_in, scalar, ps_or_sb):
            # out = (sb_in * scalar) +/- second operand, via scalar_tensor_tensor
            nc.vector.scalar_tensor_tensor(
                out_ap, in0=sb_in, scalar=scalar, in1=ps_or_sb,
                op0=AluOp.mult, op1=AluOp.add)

        YG = 2  # one out-DMA per y-pair: short tail, early PSUM drain
        for g in range(Y // YG):
            ot = opool.tile([128, TLOC, YG, Z, SC], out_dt)
            for pair in range(YG // 2):
                y = g * YG + pair * 2               # even; pair (y, y+1)
                # Phase order matches the combine's consumption order
                # (m1c needs ph1 first, bt_ ph2, t0a ph0, ...), so the DVE
                # chain starts ~1.4us earlier relative to this pair's last
                # matmul — less combine spillover past the final matmul.
                # The LAST pair ends on ph0 instead: then only t0a/ow(0)
                # are gated on the kernel's final matmul and the t=2,3
                # half-store completes before it.
                last = (g == Y // YG - 1)
                pts = [None] * NPH
                for ph in ((1, 2, 3, 4, 5, 0) if last else (1, 2, 0, 3, 4, 5)):
                    pt = ppool.tile([128, FREE], f32)
                    for aidx, (ay, az) in enumerate(OFF9):
                        rhs = ufull[:, y + ay: y + ay + 2, az: az + Z, ph, :]
                        nc.tensor.matmul(
                            pt[:],
                            wt[:, ph, aidx, :],
                            rhs,
                            start=(aidx == 0),
                            stop=(aidx == 8),
                        )
                    pts[ph] = pt
                # A^T combine; every DVE op reads at most one PSUM operand.
                # b=m1+m2, a=m1-m2, u=m3+m4, s=m3-m4
                # t0=m0+b+u; t1=a+2s; t2=b+4u; t3=a+8s+m5
                # Ordered so PSUM banks m1,m2,m0,m3,m4 free as early as
                # possible (the next pair's matmuls reuse them).
                # PSUM->SBUF copies on the Activation engine: they come off
                # the DVE critical path and overlap the DVE combines.
                m1c = tpool.tile([128, FREE], tmp_dt, tag="m1c")
                nc.scalar.copy(m1c[:], pts[1][:])
                bt_ = tpool.tile([128, FREE], tmp_dt, tag="bt")
                nc.vector.tensor_add(bt_[:], m1c[:], pts[2][:])
                t0a = tpool.tile([128, FREE], tmp_dt, tag="t0a")
                if not last:
                    nc.vector.tensor_add(t0a[:], bt_[:], pts[0][:])
                m3c = tpool.tile([128, FREE], tmp_dt, tag="m3c")
                nc.scalar.copy(m3c[:], pts[3][:])
                ut_ = tpool.tile([128, FREE], tmp_dt, tag="ut")
                nc.vector.tensor_add(ut_[:], m3c[:], pts[4][:])
                a_ = tpool.tile([128, FREE], tmp_dt, tag="at")
                nc.vector.scalar_tensor_tensor(
                    a_[:], in0=m1c[:], scalar=2.0, in1=bt_[:],
                    op0=AluOp.mult, op1=AluOp.subtract)
                s_ = tpool.tile([128, FREE], tmp_dt, tag="st")
                nc.vector.scalar_tensor_tensor(
                    s_[:], in0=m3c[:], scalar=2.0, in1=ut_[:],
                    op0=AluOp.mult, op1=AluOp.subtract)
                # writes into ot: contiguous (y2, z, sc) block at t=r
                def ow(r):
                    return ot[:, r]
                # t=0,1 first so their half-store can fire early; t3a before
                # ow(2) so ow(3) — the only op gated on phase 5's last
                # matmul — issues as soon as possible. For the last pair
                # (phase 0 computed last) t0a/ow(0) are emitted at the end
                # instead, and the t=2,3 half-store is enqueued first.
                if not last:
                    nc.vector.tensor_add(ow(0), t0a[:], ut_[:])
                stt(ow(1), s_[:], 2.0, a_[:])
                t3a = tpool.tile([128, FREE], tmp_dt, tag="t3a")
                nc.vector.scalar_tensor_tensor(
                    t3a[:], in0=s_[:], scalar=8.0, in1=a_[:],
                    op0=AluOp.mult, op1=AluOp.add)
                stt(ow(2), ut_[:], 4.0, bt_[:])
                nc.vector.tensor_add(ow(3), t3a[:], pts[5][:])
                if last:
                    # bu = b + u precomputed before phase 0's matmuls land;
                    # ow(0) = bu + ps0 is then the ONLY op after the
                    # kernel's final matmul (t0a folded away).
                    nc.vector.tensor_add(t0a[:], bt_[:], ut_[:])
                    nc.vector.tensor_add(ow(0), t0a[:], pts[0][:])
            # Contiguous half-stores per pair: the earlier-finished half
            # fires first, overlapping the rest of the combine. The last
            # pair trails only the single t=0 row behind its final matmul.
            if last:
                nc.sync.dma_start(o_dram[:, g, 2:4], ot[:, 2:4])
                nc.sync.dma_start(o_dram[:, g, 1:2], ot[:, 1:2])
                nc.sync.dma_start(o_dram[:, g, 0:1], ot[:, 0:1])
            else:
                nc.sync.dma_start(o_dram[:, g, 0:2], ot[:, 0:2])
                nc.sync.dma_start(o_dram[:, g, 2:4], ot[:, 2:4])

    # Bacc defers register allocation and sync-wait splitting to finalize();
    # run_bass_via_pjrt serializes the module as-is, so finalize here.
    nc.finalize()
    return nc


_NC_CACHE = None
LAST_RUN = None  # BassKernelResults of the most recent device run (for test.py)


def kernel(U, W, b):
    global _NC_CACHE, LAST_RUN
    shards = _prep_u_shards(np.asarray(U))
    wstat = _prep_wstat(np.asarray(W))

    if os.environ.get("CONV_EMULATE", "0") == "1":
        results = _emulate(shards, wstat)
    else:
        from concourse.bass_utils import run_bass_kernel_spmd
        if _NC_CACHE is None:
            _NC_CACHE = _build_nc()
        wr = np.ascontiguousarray(wstat.reshape(128, NPH, 9, 128))
        in_maps = [{"wstat": wr, "u": u} for u in shards]
        trace = os.environ.get("CONV_TRACE", "0") == "1"
        LAST_RUN = run_bass_kernel_spmd(
            _NC_CACHE, in_maps, core_ids=list(range(NCORES)), trace=trace)
        results = LAST_RUN.results
    return _assemble(results, np.asarray(b))


def _emulate(shards, wstat):
    """Host-side emulation of the device program (float64 accumulate)."""
    AT = np.array([
        [1, 1, 1, 1, 1, 0],
        [0, 1, -1, 2, -2, 0],
        [0, 1, 1, 4, 4, 0],
        [0, 1, -1, 8, -8, 1]], np.float64)
    results = []
    for u in shards:
        out = np.zeros((128, Y // 2, TLOC, 2, Z, SC), np.float64)
        for y in range(0, Y, 2):
            ms = []
            for ph in range(NPH):
                acc = np.zeros((128, FREE), np.float64)
                for aidx, (ay, az) in enumerate(OFF9):
                    slab = u[:, y + ay: y + ay + 2, az:az + Z, ph, :].reshape(128, -1)
                    acc += wstat[:, ph * 9 + aidx, :].T.astype(np.float64) @ slab.astype(np.float64)
                ms.append(acc.reshape(128, 2, Z, SC))
            m = np.stack(ms, axis=0)  # (6, 128, 2, Z, SC)
            res = np.einsum("rp,pnyzs->nryzs", AT, m)  # (128, 4, 2, Z, SC)
            out[:, y // 2] = res
        results.append({"out": out})
    return results

